# revision 43
# baseline (speedup 1.0000x reference)
"""Trainium2 Bass kernel for nn_Bdfdv_51170240364850 (gnn_message_passing).

Computes, for mode pairs (il, im) with im <= il (L1 = 5 modes each way) and
spatial/velocity grid (nx=1024, nv=512):

  D[il,im] = base + (-1j)*im*bx*F[il,im] + cB*bm*F[il,im+1]
             + [im==0] Re(cC*bp*F[il,1])
  base     = 0.5*bm*F[il,im-1]  (il>=1, 1<=im<=il)   else  D0[il,im]

with bx = b[:,0], bm = b[:,1]+1j b[:,2], bp = conj(bm),
cB = -(il-im)(il+im+1)/2, cC = -il(il+1).

Strategy: pure data-parallel over nx across 8 NeuronCores (nx=128 per core on
the 128 SBUF partitions), fp16 I/O, and a three-engine split:

* PE (TensorEngine): every per-x product c(x)*T runs as a diagonal-weight
  matmul accumulating in PSUM (diag(c) @ tile scales partition row p by
  c(p)).  Mode-constant coefficient parts are folded into the operands
  (P = F[im-1] + 2cB*F[im+1] fuses the set & recurrence terms; G = 2cB0*F1
  carries the im=0 coupling), so each output column needs only THREE
  matmuls and the whole kernel needs just 13 diagonal weight tiles
  (0.5b1, 1.5b1, +-0.5b2, +-m*b0, ones), which ride in with the input DMA.
* DVE: fp16 4x/2x tensor_scalar / tensor_tensor prescales (P, G) plus the
  im=0 imaginary row as two fused scalar_tensor_tensor ops (D0i as the
  fused add operand), all explicitly dependency-chained so the per-engine
  scheduler cannot starve the arrival-critical operand builds.
* ACT: drains each finished 2-bank PSUM pair (Dr,Di of one mode slot) into
  the fp16 output tile with one strided copy; the final pair drains on DVE.

Each (im, il) slot-pair is a complete 6-matmul chain on a rotating 2-bank
PSUM tile (pool bufs=4 = all 8 banks), emitted in input-arrival order so
banks never sit open.  Input DMAs are issued from the GpSimd queue (live
several us before the sync sequencer finishes its semaphore preamble) in
consumption order; outputs stream out per im-run.  DMA (fp16, ~40KB in +
28KB out per partition) is the roofline; measured ~41-43us on HW.
"""

import numpy as np

import bass_rust
import concourse.bass as bass
import concourse.tile as tile
from concourse import mybir
from concourse.bass_utils import run_bass_kernel_spmd

L1 = 5
NX = 1024
NV = 512
NCORES = 8
XS = NX // NCORES  # 128, = SBUF partitions

F32 = mybir.dt.float32
F16 = mybir.dt.float16

# ---------------------------------------------------------------------------
# slot bookkeeping (im-major ordering of the 14 valid (im, il>=1) F/D slots)
S = [(im, il) for im in range(L1) for il in range(max(1, im), L1)]
SIDX = {s: k for k, s in enumerate(S)}
NS = len(S)                      # 14
RUN = {0: 0, 1: 4, 2: 8, 3: 11, 4: 13}   # start slot index of each im-run
RL = {0: 4, 1: 4, 2: 3, 3: 2, 4: 1}      # run lengths

CB_PAIRS = [(2, 1), (3, 1), (3, 2), (4, 1), (4, 2), (4, 3)]  # (il, im)

# F/D run-interleaved layout: run m holds [re slots | im slots] back-to-back,
# so each im-run moves as ONE contiguous DMA.
FOFF = {}
_o = 0
for _m in range(L1):
    FOFF[_m] = _o
    _o += 2 * RL[_m] * NV
assert _o == 2 * NS * NV


def _cB(il, im):
    return -(il - im) * (il + im + 1) / 2.0


# pin layout (fp16): [F runs (28 NV) | D0r (4) | D0i (4) | W diags (12x128)]
WOFF = 36 * NV
NDIAG = 13      # 0.5b1, +-0.5b2, A+1..4=m*b0, A-1..4, ones, 1.5b1
DG_D1, DG_D2, DG_D3 = 0, 1, 2
DG_ONES = 11
DG_D6 = 12


def DG_AP(m):
    return 2 + m          # 3..6


def DG_AN(m):
    return 6 + m          # 7..10


CIN = WOFF + NDIAG * 128
# pscal (fp32): per-x scalars for the DVE im=0 imaginary chain
H1, H2 = 0, 1                    # 0.5*b1, 0.5*b2
NSCAL = 4
# pout layout (fp16): same run-interleaved layout as F
COUT = 2 * NS * NV


# ---------------------------------------------------------------------------
# The walrus build in this container rejects instructions carrying more than
# ONE sync-wait ("Too many sync wait commands", setupSyncWait in
# CoreV2/V3GenImpl). Tile's scheduler routinely attaches several. Post-pass:
# hoist all but the last wait of each instruction onto same-engine NOPs
# inserted immediately before it (same basic block, so per-engine program
# order is preserved).
def split_multiwaits(nc):
    for f in nc.m.functions:
        for blk in f.blocks:
            new = []
            changed = False
            for ins in blk.instructions:
                si = ins.sync_info
                if si is not None and len(si.on_wait) > 1:
                    waits = list(si.on_wait)
                    for w in waits[:-1]:
                        nop = mybir.InstNoOp(
                            name=nc.get_next_instruction_name(),
                            engine=ins.engine,
                            bass_nofuse=True,
                            sync_info=mybir.SyncInfo(on_wait=[w],
                                                     on_update=[]),
                        )
                        new.append(nop)
                    ins.sync_info = bass_rust.SyncInfo(
                        on_wait=[waits[-1]], on_update=list(si.on_update))
                    changed = True
                new.append(ins)
            if changed:
                blk.instructions = new


# ---------------------------------------------------------------------------
def _pair(ap, step_elems, nblocks=2):
    """Turn a contiguous [P, L] AP into [P, nblocks, L] with the given
    element step between blocks."""
    c = ap.copy()
    v = c.ap
    last = v.pop()
    v.append((step_elems, nblocks))
    v.append(tuple(last))
    c.ap = v
    return c


def build_bass(split=True):
    MULT = mybir.AluOpType.mult
    ADD = mybir.AluOpType.add

    nc = bass.Bass()
    pin = nc.dram_tensor("pin", [XS, CIN], F16, kind="ExternalInput").ap()
    pscal = nc.dram_tensor("pscal", [XS, NSCAL], F16,
                           kind="ExternalInput").ap()
    pout = nc.dram_tensor("pout", [XS, COUT], F16, kind="ExternalOutput").ap()

    with tile.TileContext(nc) as tc:
        with tc.tile_pool(name="m", bufs=1) as pool, \
             tc.psum_pool(name="p", bufs=4) as ppool:
            fF = pool.tile([XS, 2 * NS * NV], F16, tag="fF")
            fD0 = pool.tile([XS, 8 * NV], F16, tag="fD0")
            fW = pool.tile([XS, NDIAG * 128], F16, tag="fW")
            scal = pool.tile([XS, NSCAL], F16, tag="scal")
            P = pool.tile([XS, 2 * 6 * NV], F16, tag="P")
            G = pool.tile([XS, 2 * 4 * NV], F16, tag="G")
            OUT = pool.tile([XS, 2 * NS * NV], F16, tag="OUT")

            def fslot(k, imag, n=1):
                m = S[k][0]
                o = FOFF[m] + (imag * RL[m] + (k - RUN[m])) * NV
                return fF[:, o:o + n * NV]

            def fr(k):
                return fslot(k, 0)

            def fi(k):
                return fslot(k, 1)

            def pr(j):
                return P[:, j * NV:(j + 1) * NV]

            def pi(j):
                return P[:, (6 + j) * NV:(7 + j) * NV]

            def W(j):
                return fW[:, j * 128:(j + 1) * 128]

            def outr(k, n=1):
                m = S[k][0]
                o = FOFF[m] + (k - RUN[m]) * NV
                return OUT[:, o:o + n * NV]

            def outi(k, n=1):
                m = S[k][0]
                o = FOFF[m] + (RL[m] + k - RUN[m]) * NV
                return OUT[:, o:o + n * NV]

            def sc(col):
                return scal[:, col:col + 1]

            # ---- input DMAs: issued from the GpSimd queue, which is live
            # ~6us before the sync sequencer finishes its preamble; FIFO
            # drain makes emission order the arrival priority.
            nc.gpsimd.dma_start(scal[:], pscal[:])
            nc.gpsimd.dma_start(fW[:], pin[:, WOFF:WOFF + NDIAG * 128])

            def in_run(m):
                o = FOFF[m]
                n = 2 * RL[m] * NV
                nc.gpsimd.dma_start(fF[:, o:o + n], pin[:, o:o + n])

            in_run(0)
            in_run(1)
            in_run(2)
            nc.gpsimd.dma_start(fD0[:], pin[:, 28 * NV:36 * NV])
            in_run(3)
            in_run(4)

            # ---- DVE prescales ----
            def presc_G(il):        # (Gr,Gi) = 2*cB0(il) * (Fr1,Fi1)
                k1 = SIDX[(1, il)]
                return nc.vector.tensor_scalar_mul(
                    _pair(G[:, (il - 1) * NV:il * NV], 4 * NV),
                    _pair(fr(k1), RL[1] * NV),
                    float(-il * (il + 1)))

            def presc_SF(j):        # P = 2cB * F[im+1]   (fp16 TS at 4x)
                il, im = CB_PAIRS[j]
                ks = SIDX[(im + 1, il)]
                return nc.vector.tensor_scalar_mul(
                    _pair(pr(j), 6 * NV),
                    _pair(fr(ks), RL[im + 1] * NV),
                    2.0 * _cB(il, im))

            def presc_P(j):         # P += F[im-1]          (fp16 TT at 2x)
                il, im = CB_PAIRS[j]
                kb = SIDX[(im - 1, il)]
                return nc.vector.tensor_tensor(
                    _pair(pr(j), 6 * NV),
                    _pair(pr(j), 6 * NV),
                    _pair(fr(kb), RL[im - 1] * NV),
                    ADD)

            # Tile's per-engine scheduler reorders by readiness, which can
            # push the group-3 P operand behind the long im=0 chain.  Chain
            # the DVE ops explicitly so arrival-critical prescales run in
            # priority order.
            from bass_rust import add_dep_helper
            _dve_prev = None

            def dve_chain(ins):
                nonlocal _dve_prev
                if _dve_prev is not None:
                    add_dep_helper(ins.ins, _dve_prev.ins,
                                   reason="DVE priority order")
                _dve_prev = ins
                return ins

            gr = G[:, 0:4 * NV]
            gi = G[:, 4 * NV:8 * NV]
            d0i = fD0[:, 4 * NV:8 * NV]
            for il in range(1, L1):
                dve_chain(presc_G(il))  # needs run 1; feeds the b0 PE chain
            dve_chain(nc.vector.scalar_tensor_tensor(  # Di0 = D0i+0.5b1*Gi
                outi(0, 4), gi, sc(H1), d0i, MULT, ADD))
            for j in (0, 1, 3):     # group-1 operands (need runs 0 & 2)
                dve_chain(presc_SF(j))
                dve_chain(presc_P(j))
            for j in (2, 4):        # group-2 (runs 1 & 3)
                dve_chain(presc_SF(j))
                dve_chain(presc_P(j))
            dve_chain(presc_SF(5))  # group-3 (runs 2 & 4)
            dve_chain(presc_P(5))
            dve_chain(nc.vector.scalar_tensor_tensor(  # ... + 0.5b2*Gr
                outi(0, 4), gr, sc(H2), outi(0, 4), MULT, ADD))

            # ---- PE: pair-level pipeline.  Each (g, il) slot-pair is one
            # 2-bank PSUM tile and a complete 6-matmul chain that closes
            # immediately, so banks never sit open waiting for late operands
            # and the ACT evacuation drain starts ~15us earlier.  Emission
            # order = data-arrival order: diagonal pairs (no P operand)
            # first, then im=0 real row, then the P-merged middle pairs.
            def mm(bank, j, rhs, start=False, stop=False):
                nc.tensor.matmul(bank, W(j), rhs, start=start, stop=stop,
                                 skip_group_check=True)

            def pair_tile(name):
                return ppool.tile([XS, 2 * NV], F32, tag="pk", name=name)

            def pout_pair(k):
                m = S[k][0]
                o = FOFF[m] + (k - RUN[m]) * NV
                return _pair(pout[:, o:o + NV], RL[m] * NV)

            def diag_pair(g):
                pk = pair_tile(f"d{g}")
                bR, bI = pk[:, 0:NV], pk[:, NV:2 * NV]
                kp, ks = SIDX[(g - 1, g)], SIDX[(g, g)]
                mm(bR, DG_D1, fr(kp), start=True)
                mm(bI, DG_D1, fi(kp), start=True)
                mm(bR, DG_D3, fi(kp))
                mm(bI, DG_D2, fr(kp))
                mm(bR, DG_AP(g), fi(ks), stop=True)
                mm(bI, DG_AN(g), fr(ks), stop=True)
                nc.scalar.copy(_pair(outr(ks), RL[g] * NV), pk[:])
                nc.sync.dma_start(pout_pair(ks), _pair(outr(ks), RL[g] * NV))

            def mid_pair(il, g, evac="act"):
                pk = pair_tile(f"m{il}{g}")
                bR, bI = pk[:, 0:NV], pk[:, NV:2 * NV]
                ks = SIDX[(g, il)]
                j = CB_PAIRS.index((il, g))
                mm(bR, DG_AP(g), fi(ks), start=True)
                mm(bI, DG_AN(g), fr(ks), start=True)
                mm(bR, DG_D1, pr(j))
                mm(bI, DG_D1, pi(j))
                mm(bR, DG_D3, pi(j), stop=True)
                mm(bI, DG_D2, pr(j), stop=True)
                dst = _pair(outr(ks), RL[g] * NV)
                if evac == "act":
                    nc.scalar.copy(dst, pk[:])
                else:
                    dve_chain(nc.vector.tensor_copy(dst, pk[:]))
                nc.sync.dma_start(pout_pair(ks), dst)

            def b0_pairs():
                # im=0 real row: Dr0 = 1.5b1@Gr + 0.5b2@Gi + ones@D0r
                b0p = [pair_tile("b0a"), pair_tile("b0b")]

                def bk(il):
                    return b0p[(il - 1) // 2][:, ((il - 1) % 2) * NV:
                                              ((il - 1) % 2 + 1) * NV]

                for il in range(1, L1):
                    mm(bk(il), DG_D6, G[:, (il - 1) * NV:il * NV],
                       start=True)
                for il in range(1, L1):
                    mm(bk(il), DG_D2, G[:, (3 + il) * NV:(4 + il) * NV])
                for il in range(1, L1):
                    mm(bk(il), DG_ONES, fD0[:, (il - 1) * NV:il * NV],
                       stop=True)
                nc.scalar.copy(OUT[:, 0:2 * NV], b0p[0][:])
                nc.sync.dma_start(pout[:, 0:2 * NV], OUT[:, 0:2 * NV])
                nc.scalar.copy(OUT[:, 2 * NV:4 * NV], b0p[1][:])
                nc.sync.dma_start(pout[:, 2 * NV:4 * NV],
                                  OUT[:, 2 * NV:4 * NV])

            def out_run(m):
                o = FOFF[m]
                n = 2 * RL[m] * NV
                nc.sync.dma_start(pout[:, o:o + n], OUT[:, o:o + n])

            # PE warm-up: throwaway matmuls gated only on the early fW DMA
            # keep the HAM activity window busy so the real chains issue at
            # the warm 2.4GHz clock instead of the cold 1.2GHz default.
            warm = pair_tile("warm")
            for _ in range(20):
                mm(warm[:, 0:256], DG_D1, fW[:, 0:256], start=True,
                   stop=True)

            diag_pair(1)
            diag_pair(2)
            diag_pair(3)
            b0_pairs()
            mid_pair(2, 1)
            mid_pair(3, 1)
            mid_pair(4, 1)
            diag_pair(4)
            mid_pair(3, 2)
            mid_pair(4, 2)
            mid_pair(4, 3, "dve")
            # im=0 imaginary half, written by the DVE chain
            nc.sync.dma_start(pout[:, 4 * NV:8 * NV], OUT[:, 4 * NV:8 * NV])

    if split:
        split_multiwaits(nc)
    return nc


# ---------------------------------------------------------------------------
def pack_inputs(prev_f_re, prev_f_im, delta0_re, delta0_im, b):
    """-> list of per-core {'pin': [XS, CIN] f16, 'pscal': [XS, 4] f32}."""
    pr = np.asarray(prev_f_re, np.float32)
    pi = np.asarray(prev_f_im, np.float32)
    d0r = np.asarray(delta0_re, np.float32)
    d0i = np.asarray(delta0_im, np.float32)
    bb = np.asarray(b, np.float32)
    ar = np.arange(XS)
    in_maps = []
    for c in range(NCORES):
        X = slice(c * XS, (c + 1) * XS)
        p = np.zeros((XS, CIN), np.float16)
        for k, (im, il) in enumerate(S):
            o = FOFF[im] + (k - RUN[im]) * NV
            p[:, o:o + NV] = pr[il, im, X, :]
            o += RL[im] * NV
            p[:, o:o + NV] = pi[il, im, X, :]
        for il in range(1, L1):
            p[:, (28 + il - 1) * NV:(29 + il - 1) * NV] = d0r[il, 0, X, :]
            p[:, (32 + il - 1) * NV:(33 + il - 1) * NV] = d0i[il, 0, X, :]
        b0, b1, b2 = bb[X, 0], bb[X, 1], bb[X, 2]
        diags = [0.5 * b1, 0.5 * b2, -0.5 * b2,
                 1.0 * b0, 2.0 * b0, 3.0 * b0, 4.0 * b0,
                 -1.0 * b0, -2.0 * b0, -3.0 * b0, -4.0 * b0,
                 np.ones(XS, np.float32), 1.5 * b1]
        for j, cx in enumerate(diags):
            blk = np.zeros((XS, 128), np.float16)
            blk[ar, ar] = cx.astype(np.float16)
            p[:, WOFF + j * 128:WOFF + (j + 1) * 128] = blk
        ps = np.zeros((XS, NSCAL), np.float16)
        ps[:, H1] = 0.5 * b1
        ps[:, H2] = 0.5 * b2
        in_maps.append({"pin": p, "pscal": ps})
    return in_maps


def unpack_outputs(results, delta0_re, delta0_im):
    out = np.zeros((L1, L1, NX, NV), np.complex64)
    out[0, 0] = np.asarray(delta0_re[0, 0]) + 1j * np.asarray(delta0_im[0, 0])
    for c in range(NCORES):
        X = slice(c * XS, (c + 1) * XS)
        p = results[c]["pout"]
        for k, (im, il) in enumerate(S):
            o = FOFF[im] + (k - RUN[im]) * NV
            dr = p[:, o:o + NV].astype(np.float32)
            o += RL[im] * NV
            di = p[:, o:o + NV].astype(np.float32)
            out[il, im, X, :] = dr + 1j * di
    return out


_NC_CACHE = None


def get_nc():
    global _NC_CACHE
    if _NC_CACHE is None:
        _NC_CACHE = build_bass()
    return _NC_CACHE


def kernel(prev_f_re, prev_f_im, delta0_re, delta0_im, b, v):
    in_maps = pack_inputs(prev_f_re, prev_f_im, delta0_re, delta0_im, b)
    res = run_bass_kernel_spmd(get_nc(), in_maps, list(range(NCORES)))
    return unpack_outputs(res.results, delta0_re, delta0_im)


# revision 44
# speedup vs baseline: 1.0202x; 1.0202x over previous
"""Trainium2 Bass kernel for nn_Bdfdv_51170240364850 (gnn_message_passing).

Computes, for mode pairs (il, im) with im <= il (L1 = 5 modes each way) and
spatial/velocity grid (nx=1024, nv=512):

  D[il,im] = base + (-1j)*im*bx*F[il,im] + cB*bm*F[il,im+1]
             + [im==0] Re(cC*bp*F[il,1])
  base     = 0.5*bm*F[il,im-1]  (il>=1, 1<=im<=il)   else  D0[il,im]

with bx = b[:,0], bm = b[:,1]+1j b[:,2], bp = conj(bm),
cB = -(il-im)(il+im+1)/2, cC = -il(il+1).

Strategy: pure data-parallel over nx across 8 NeuronCores (nx=128 per core on
the 128 SBUF partitions), fp16 I/O, and a three-engine split:

* PE (TensorEngine): every per-x product c(x)*T runs as a diagonal-weight
  matmul accumulating in PSUM (diag(c) @ tile scales partition row p by
  c(p)).  Mode-constant coefficient parts are folded into the operands
  (P = F[im-1] + 2cB*F[im+1] fuses the set & recurrence terms; G = 2cB0*F1
  carries the im=0 coupling), so each output column needs only THREE
  matmuls and the whole kernel needs just 13 diagonal weight tiles
  (0.5b1, 1.5b1, +-0.5b2, +-m*b0, ones), which ride in with the input DMA.
* DVE: fp16 4x/2x tensor_scalar / tensor_tensor prescales (P, G) plus the
  im=0 imaginary row as two fused scalar_tensor_tensor ops (D0i as the
  fused add operand), all explicitly dependency-chained so the per-engine
  scheduler cannot starve the arrival-critical operand builds.
* ACT: drains each finished 2-bank PSUM pair (Dr,Di of one mode slot) into
  the fp16 output tile with one strided copy; the final pair drains on DVE.

Each (im, il) slot-pair is a complete 6-matmul chain on a rotating 2-bank
PSUM tile (pool bufs=4 = all 8 banks), emitted in input-arrival order so
banks never sit open.  Input DMAs are issued from the GpSimd queue (live
several us before the sync sequencer finishes its semaphore preamble) in
consumption order; outputs stream out per im-run.  DMA (fp16, ~40KB in +
28KB out per partition) is the roofline; measured ~41-43us on HW.
"""

import numpy as np

import bass_rust
import concourse.bass as bass
import concourse.tile as tile
from concourse import mybir
from concourse.bass_utils import run_bass_kernel_spmd

L1 = 5
NX = 1024
NV = 512
NCORES = 8
XS = NX // NCORES  # 128, = SBUF partitions

F32 = mybir.dt.float32
F16 = mybir.dt.float16

# ---------------------------------------------------------------------------
# slot bookkeeping (im-major ordering of the 14 valid (im, il>=1) F/D slots)
S = [(im, il) for im in range(L1) for il in range(max(1, im), L1)]
SIDX = {s: k for k, s in enumerate(S)}
NS = len(S)                      # 14
RUN = {0: 0, 1: 4, 2: 8, 3: 11, 4: 13}   # start slot index of each im-run
RL = {0: 4, 1: 4, 2: 3, 3: 2, 4: 1}      # run lengths

CB_PAIRS = [(2, 1), (3, 1), (3, 2), (4, 1), (4, 2), (4, 3)]  # (il, im)

# F/D run-interleaved layout: run m holds [re slots | im slots] back-to-back,
# so each im-run moves as ONE contiguous DMA.
FOFF = {}
_o = 0
for _m in range(L1):
    FOFF[_m] = _o
    _o += 2 * RL[_m] * NV
assert _o == 2 * NS * NV


def _cB(il, im):
    return -(il - im) * (il + im + 1) / 2.0


# pin layout (fp16): [F runs (28 NV) | D0r (4) | D0i (4) | W diags (12x128)]
WOFF = 36 * NV
NDIAG = 13      # 0.5b1, +-0.5b2, A+1..4=m*b0, A-1..4, ones, 1.5b1
DG_D1, DG_D2, DG_D3 = 0, 1, 2
DG_ONES = 11
DG_D6 = 12


def DG_AP(m):
    return 2 + m          # 3..6


def DG_AN(m):
    return 6 + m          # 7..10


CIN = WOFF + NDIAG * 128
# pscal (fp32): per-x scalars for the DVE im=0 imaginary chain
H1, H2 = 0, 1                    # 0.5*b1, 0.5*b2
NSCAL = 4
# pout layout (fp16): same run-interleaved layout as F
COUT = 2 * NS * NV


# ---------------------------------------------------------------------------
# The walrus build in this container rejects instructions carrying more than
# ONE sync-wait ("Too many sync wait commands", setupSyncWait in
# CoreV2/V3GenImpl). Tile's scheduler routinely attaches several. Post-pass:
# hoist all but the last wait of each instruction onto same-engine NOPs
# inserted immediately before it (same basic block, so per-engine program
# order is preserved).
def split_multiwaits(nc):
    for f in nc.m.functions:
        for blk in f.blocks:
            new = []
            changed = False
            for ins in blk.instructions:
                si = ins.sync_info
                if si is not None and len(si.on_wait) > 1:
                    waits = list(si.on_wait)
                    for w in waits[:-1]:
                        nop = mybir.InstNoOp(
                            name=nc.get_next_instruction_name(),
                            engine=ins.engine,
                            bass_nofuse=True,
                            sync_info=mybir.SyncInfo(on_wait=[w],
                                                     on_update=[]),
                        )
                        new.append(nop)
                    ins.sync_info = bass_rust.SyncInfo(
                        on_wait=[waits[-1]], on_update=list(si.on_update))
                    changed = True
                new.append(ins)
            if changed:
                blk.instructions = new


# ---------------------------------------------------------------------------
def _pair(ap, step_elems, nblocks=2):
    """Turn a contiguous [P, L] AP into [P, nblocks, L] with the given
    element step between blocks."""
    c = ap.copy()
    v = c.ap
    last = v.pop()
    v.append((step_elems, nblocks))
    v.append(tuple(last))
    c.ap = v
    return c


def build_bass(split=True):
    MULT = mybir.AluOpType.mult
    ADD = mybir.AluOpType.add

    nc = bass.Bass()
    pin = nc.dram_tensor("pin", [XS, CIN], F16, kind="ExternalInput").ap()
    pscal = nc.dram_tensor("pscal", [XS, NSCAL], F16,
                           kind="ExternalInput").ap()
    pout = nc.dram_tensor("pout", [XS, COUT], F16, kind="ExternalOutput").ap()

    with tile.TileContext(nc) as tc:
        with tc.tile_pool(name="m", bufs=1) as pool, \
             tc.psum_pool(name="p", bufs=4) as ppool:
            fF = pool.tile([XS, 2 * NS * NV], F16, tag="fF")
            fD0 = pool.tile([XS, 8 * NV], F16, tag="fD0")
            fW = pool.tile([XS, NDIAG * 128], F16, tag="fW")
            scal = pool.tile([XS, NSCAL], F16, tag="scal")
            P = pool.tile([XS, 2 * 6 * NV], F16, tag="P")
            G = pool.tile([XS, 2 * 4 * NV], F16, tag="G")
            OUT = pool.tile([XS, 2 * NS * NV], F16, tag="OUT")

            def fslot(k, imag, n=1):
                m = S[k][0]
                o = FOFF[m] + (imag * RL[m] + (k - RUN[m])) * NV
                return fF[:, o:o + n * NV]

            def fr(k):
                return fslot(k, 0)

            def fi(k):
                return fslot(k, 1)

            def pr(j):
                return P[:, j * NV:(j + 1) * NV]

            def pi(j):
                return P[:, (6 + j) * NV:(7 + j) * NV]

            def W(j):
                return fW[:, j * 128:(j + 1) * 128]

            def outr(k, n=1):
                m = S[k][0]
                o = FOFF[m] + (k - RUN[m]) * NV
                return OUT[:, o:o + n * NV]

            def outi(k, n=1):
                m = S[k][0]
                o = FOFF[m] + (RL[m] + k - RUN[m]) * NV
                return OUT[:, o:o + n * NV]

            def sc(col):
                return scal[:, col:col + 1]

            # ---- input DMAs: issued from the GpSimd queue, which is live
            # ~6us before the sync sequencer finishes its preamble; FIFO
            # drain makes emission order the arrival priority.
            # tiny scal table rides the sync queue (live by ~8.6us, needed
            # ~19us) so it doesn't burn a Q7 descriptor-gen slot at the head
            # of the input stream
            nc.sync.dma_start(scal[:], pscal[:])
            nc.gpsimd.dma_start(fW[:], pin[:, WOFF:WOFF + NDIAG * 128])

            def in_run(m):
                o = FOFF[m]
                n = 2 * RL[m] * NV
                nc.gpsimd.dma_start(fF[:, o:o + n], pin[:, o:o + n])

            in_run(0)
            in_run(1)
            in_run(2)
            nc.gpsimd.dma_start(fD0[:], pin[:, 28 * NV:36 * NV])
            in_run(3)
            in_run(4)

            # ---- DVE prescales ----
            def presc_G(il):        # (Gr,Gi) = 2*cB0(il) * (Fr1,Fi1)
                k1 = SIDX[(1, il)]
                return nc.vector.tensor_scalar_mul(
                    _pair(G[:, (il - 1) * NV:il * NV], 4 * NV),
                    _pair(fr(k1), RL[1] * NV),
                    float(-il * (il + 1)))

            def presc_SF(j):        # P = 2cB * F[im+1]   (fp16 TS at 4x)
                il, im = CB_PAIRS[j]
                ks = SIDX[(im + 1, il)]
                return nc.vector.tensor_scalar_mul(
                    _pair(pr(j), 6 * NV),
                    _pair(fr(ks), RL[im + 1] * NV),
                    2.0 * _cB(il, im))

            def presc_P(j):         # P += F[im-1]          (fp16 TT at 2x)
                il, im = CB_PAIRS[j]
                kb = SIDX[(im - 1, il)]
                return nc.vector.tensor_tensor(
                    _pair(pr(j), 6 * NV),
                    _pair(pr(j), 6 * NV),
                    _pair(fr(kb), RL[im - 1] * NV),
                    ADD)

            # Tile's per-engine scheduler reorders by readiness, which can
            # push the group-3 P operand behind the long im=0 chain.  Chain
            # the DVE ops explicitly so arrival-critical prescales run in
            # priority order.
            from bass_rust import add_dep_helper
            _dve_prev = None

            def dve_chain(ins):
                nonlocal _dve_prev
                if _dve_prev is not None:
                    add_dep_helper(ins.ins, _dve_prev.ins,
                                   reason="DVE priority order")
                _dve_prev = ins
                return ins

            gr = G[:, 0:4 * NV]
            gi = G[:, 4 * NV:8 * NV]
            d0i = fD0[:, 4 * NV:8 * NV]
            for il in range(1, L1):
                dve_chain(presc_G(il))  # needs run 1; feeds the b0 PE chain
            dve_chain(nc.vector.scalar_tensor_tensor(  # Di0 = D0i+0.5b1*Gi
                outi(0, 4), gi, sc(H1), d0i, MULT, ADD))
            for j in (0, 1, 3):     # group-1 operands (need runs 0 & 2)
                dve_chain(presc_SF(j))
                dve_chain(presc_P(j))
            for j in (2, 4):        # group-2 (runs 1 & 3)
                dve_chain(presc_SF(j))
                dve_chain(presc_P(j))
            dve_chain(presc_SF(5))  # group-3 (runs 2 & 4)
            dve_chain(presc_P(5))
            dve_chain(nc.vector.scalar_tensor_tensor(  # ... + 0.5b2*Gr
                outi(0, 4), gr, sc(H2), outi(0, 4), MULT, ADD))

            # ---- PE: pair-level pipeline.  Each (g, il) slot-pair is one
            # 2-bank PSUM tile and a complete 6-matmul chain that closes
            # immediately, so banks never sit open waiting for late operands
            # and the ACT evacuation drain starts ~15us earlier.  Emission
            # order = data-arrival order: diagonal pairs (no P operand)
            # first, then im=0 real row, then the P-merged middle pairs.
            def mm(bank, j, rhs, start=False, stop=False):
                nc.tensor.matmul(bank, W(j), rhs, start=start, stop=stop,
                                 skip_group_check=True)

            def pair_tile(name):
                return ppool.tile([XS, 2 * NV], F32, tag="pk", name=name)

            def pout_pair(k):
                m = S[k][0]
                o = FOFF[m] + (k - RUN[m]) * NV
                return _pair(pout[:, o:o + NV], RL[m] * NV)

            def diag_pair(g):
                pk = pair_tile(f"d{g}")
                bR, bI = pk[:, 0:NV], pk[:, NV:2 * NV]
                kp, ks = SIDX[(g - 1, g)], SIDX[(g, g)]
                mm(bR, DG_D1, fr(kp), start=True)
                mm(bI, DG_D1, fi(kp), start=True)
                mm(bR, DG_D3, fi(kp))
                mm(bI, DG_D2, fr(kp))
                mm(bR, DG_AP(g), fi(ks), stop=True)
                mm(bI, DG_AN(g), fr(ks), stop=True)
                nc.scalar.copy(_pair(outr(ks), RL[g] * NV), pk[:])
                nc.sync.dma_start(pout_pair(ks), _pair(outr(ks), RL[g] * NV))

            def mid_pair(il, g, evac="act"):
                pk = pair_tile(f"m{il}{g}")
                bR, bI = pk[:, 0:NV], pk[:, NV:2 * NV]
                ks = SIDX[(g, il)]
                j = CB_PAIRS.index((il, g))
                mm(bR, DG_AP(g), fi(ks), start=True)
                mm(bI, DG_AN(g), fr(ks), start=True)
                mm(bR, DG_D1, pr(j))
                mm(bI, DG_D1, pi(j))
                mm(bR, DG_D3, pi(j), stop=True)
                mm(bI, DG_D2, pr(j), stop=True)
                dst = _pair(outr(ks), RL[g] * NV)
                if evac == "act":
                    nc.scalar.copy(dst, pk[:])
                else:
                    dve_chain(nc.vector.tensor_copy(dst, pk[:]))
                nc.sync.dma_start(pout_pair(ks), dst)

            def b0_pairs():
                # im=0 real row: Dr0 = 1.5b1@Gr + 0.5b2@Gi + ones@D0r
                b0p = [pair_tile("b0a"), pair_tile("b0b")]

                def bk(il):
                    return b0p[(il - 1) // 2][:, ((il - 1) % 2) * NV:
                                              ((il - 1) % 2 + 1) * NV]

                for il in range(1, L1):
                    mm(bk(il), DG_D6, G[:, (il - 1) * NV:il * NV],
                       start=True)
                for il in range(1, L1):
                    mm(bk(il), DG_D2, G[:, (3 + il) * NV:(4 + il) * NV])
                for il in range(1, L1):
                    mm(bk(il), DG_ONES, fD0[:, (il - 1) * NV:il * NV],
                       stop=True)
                nc.scalar.copy(OUT[:, 0:2 * NV], b0p[0][:])
                nc.sync.dma_start(pout[:, 0:2 * NV], OUT[:, 0:2 * NV])
                nc.scalar.copy(OUT[:, 2 * NV:4 * NV], b0p[1][:])
                nc.sync.dma_start(pout[:, 2 * NV:4 * NV],
                                  OUT[:, 2 * NV:4 * NV])

            def out_run(m):
                o = FOFF[m]
                n = 2 * RL[m] * NV
                nc.sync.dma_start(pout[:, o:o + n], OUT[:, o:o + n])

            # PE warm-up: throwaway matmuls gated only on the early fW DMA
            # keep the HAM activity window busy so the real chains issue at
            # the warm 2.4GHz clock instead of the cold 1.2GHz default.
            warm = pair_tile("warm")
            for _ in range(20):
                mm(warm[:, 0:256], DG_D1, fW[:, 0:256], start=True,
                   stop=True)

            diag_pair(1)
            diag_pair(2)
            diag_pair(3)
            b0_pairs()
            mid_pair(2, 1)
            mid_pair(3, 1)
            mid_pair(4, 1)
            diag_pair(4)
            mid_pair(3, 2)
            mid_pair(4, 2)
            mid_pair(4, 3, "dve")
            # im=0 imaginary half, written by the DVE chain
            nc.sync.dma_start(pout[:, 4 * NV:8 * NV], OUT[:, 4 * NV:8 * NV])

    if split:
        split_multiwaits(nc)
    return nc


# ---------------------------------------------------------------------------
def pack_inputs(prev_f_re, prev_f_im, delta0_re, delta0_im, b):
    """-> list of per-core {'pin': [XS, CIN] f16, 'pscal': [XS, 4] f32}."""
    pr = np.asarray(prev_f_re, np.float32)
    pi = np.asarray(prev_f_im, np.float32)
    d0r = np.asarray(delta0_re, np.float32)
    d0i = np.asarray(delta0_im, np.float32)
    bb = np.asarray(b, np.float32)
    ar = np.arange(XS)
    in_maps = []
    for c in range(NCORES):
        X = slice(c * XS, (c + 1) * XS)
        p = np.zeros((XS, CIN), np.float16)
        for k, (im, il) in enumerate(S):
            o = FOFF[im] + (k - RUN[im]) * NV
            p[:, o:o + NV] = pr[il, im, X, :]
            o += RL[im] * NV
            p[:, o:o + NV] = pi[il, im, X, :]
        for il in range(1, L1):
            p[:, (28 + il - 1) * NV:(29 + il - 1) * NV] = d0r[il, 0, X, :]
            p[:, (32 + il - 1) * NV:(33 + il - 1) * NV] = d0i[il, 0, X, :]
        b0, b1, b2 = bb[X, 0], bb[X, 1], bb[X, 2]
        diags = [0.5 * b1, 0.5 * b2, -0.5 * b2,
                 1.0 * b0, 2.0 * b0, 3.0 * b0, 4.0 * b0,
                 -1.0 * b0, -2.0 * b0, -3.0 * b0, -4.0 * b0,
                 np.ones(XS, np.float32), 1.5 * b1]
        for j, cx in enumerate(diags):
            blk = np.zeros((XS, 128), np.float16)
            blk[ar, ar] = cx.astype(np.float16)
            p[:, WOFF + j * 128:WOFF + (j + 1) * 128] = blk
        ps = np.zeros((XS, NSCAL), np.float16)
        ps[:, H1] = 0.5 * b1
        ps[:, H2] = 0.5 * b2
        in_maps.append({"pin": p, "pscal": ps})
    return in_maps


def unpack_outputs(results, delta0_re, delta0_im):
    out = np.zeros((L1, L1, NX, NV), np.complex64)
    out[0, 0] = np.asarray(delta0_re[0, 0]) + 1j * np.asarray(delta0_im[0, 0])
    for c in range(NCORES):
        X = slice(c * XS, (c + 1) * XS)
        p = results[c]["pout"]
        for k, (im, il) in enumerate(S):
            o = FOFF[im] + (k - RUN[im]) * NV
            dr = p[:, o:o + NV].astype(np.float32)
            o += RL[im] * NV
            di = p[:, o:o + NV].astype(np.float32)
            out[il, im, X, :] = dr + 1j * di
    return out


_NC_CACHE = None


def get_nc():
    global _NC_CACHE
    if _NC_CACHE is None:
        _NC_CACHE = build_bass()
    return _NC_CACHE


def kernel(prev_f_re, prev_f_im, delta0_re, delta0_im, b, v):
    in_maps = pack_inputs(prev_f_re, prev_f_im, delta0_re, delta0_im, b)
    res = run_bass_kernel_spmd(get_nc(), in_maps, list(range(NCORES)))
    return unpack_outputs(res.results, delta0_re, delta0_im)


# revision 45
# speedup vs baseline: 1.1021x; 1.0802x over previous
"""Trainium2 Bass kernel for nn_Bdfdv_51170240364850 (gnn_message_passing).

Computes, for mode pairs (il, im) with im <= il (L1 = 5 modes each way) and
spatial/velocity grid (nx=1024, nv=512):

  D[il,im] = base + (-1j)*im*bx*F[il,im] + cB*bm*F[il,im+1]
             + [im==0] Re(cC*bp*F[il,1])
  base     = 0.5*bm*F[il,im-1]  (il>=1, 1<=im<=il)   else  D0[il,im]

with bx = b[:,0], bm = b[:,1]+1j b[:,2], bp = conj(bm),
cB = -(il-im)(il+im+1)/2, cC = -il(il+1).

Strategy: pure data-parallel over nx across 8 NeuronCores (nx=128 per core on
the 128 SBUF partitions), fp16 I/O, and a three-engine split:

* PE (TensorEngine): every per-x product c(x)*T runs as a diagonal-weight
  matmul accumulating in PSUM (diag(c) @ tile scales partition row p by
  c(p)).  Mode-constant coefficient parts are folded into the operands
  (P = F[im-1] + 2cB*F[im+1] fuses the set & recurrence terms; G = 2cB0*F1
  carries the im=0 coupling), so each output column needs only THREE
  matmuls and the whole kernel needs just 13 diagonal weight tiles
  (0.5b1, 1.5b1, +-0.5b2, +-m*b0, ones), which ride in with the input DMA.
* DVE: fp16 4x/2x tensor_scalar / tensor_tensor prescales (P, G) plus the
  im=0 imaginary row as two fused scalar_tensor_tensor ops (D0i as the
  fused add operand), all explicitly dependency-chained so the per-engine
  scheduler cannot starve the arrival-critical operand builds.
* ACT: drains each finished 2-bank PSUM pair (Dr,Di of one mode slot) into
  the fp16 output tile with one strided copy; the final pair drains on DVE.

Each (im, il) slot-pair is a complete 6-matmul chain on a rotating 2-bank
PSUM tile (pool bufs=4 = all 8 banks), emitted in input-arrival order so
banks never sit open.  Input DMAs are issued from the GpSimd queue (live
several us before the sync sequencer finishes its semaphore preamble) in
consumption order; outputs stream out per im-run.  DMA (fp16, ~40KB in +
28KB out per partition) is the roofline; measured ~41-43us on HW.
"""

import numpy as np

import bass_rust
import concourse.bass as bass
import concourse.tile as tile
from concourse import mybir
from concourse.bass_utils import run_bass_kernel_spmd

L1 = 5
NX = 1024
NV = 512
NCORES = 8
XS = NX // NCORES  # 128, = SBUF partitions

F32 = mybir.dt.float32
F16 = mybir.dt.float16

# ---------------------------------------------------------------------------
# slot bookkeeping (im-major ordering of the 14 valid (im, il>=1) F/D slots)
S = [(im, il) for im in range(L1) for il in range(max(1, im), L1)]
SIDX = {s: k for k, s in enumerate(S)}
NS = len(S)                      # 14
RUN = {0: 0, 1: 4, 2: 8, 3: 11, 4: 13}   # start slot index of each im-run
RL = {0: 4, 1: 4, 2: 3, 3: 2, 4: 1}      # run lengths

CB_PAIRS = [(2, 1), (3, 1), (3, 2), (4, 1), (4, 2), (4, 3)]  # (il, im)

# F/D run-interleaved layout: run m holds [re slots | im slots] back-to-back,
# so each im-run moves as ONE contiguous DMA.
FOFF = {}
_o = 0
for _m in range(L1):
    FOFF[_m] = _o
    _o += 2 * RL[_m] * NV
assert _o == 2 * NS * NV


def _cB(il, im):
    return -(il - im) * (il + im + 1) / 2.0


# pin layout (fp16): [F runs (28 NV) | D0r (4) | D0i (4) | W diags (12x128)]
WOFF = 36 * NV
NDIAG = 13      # 0.5b1, +-0.5b2, A+1..4=m*b0, A-1..4, ones, 1.5b1
DG_D1, DG_D2, DG_D3 = 0, 1, 2
DG_ONES = 11
DG_D6 = 12


def DG_AP(m):
    return 2 + m          # 3..6


def DG_AN(m):
    return 6 + m          # 7..10


CIN = WOFF + NDIAG * 128
# pscal (fp32): per-x scalars for the DVE im=0 imaginary chain
H1, H2 = 0, 1                    # 0.5*b1, 0.5*b2
NSCAL = 4
# pout layout (fp16): same run-interleaved layout as F
COUT = 2 * NS * NV


# ---------------------------------------------------------------------------
# The walrus build in this container rejects instructions carrying more than
# ONE sync-wait ("Too many sync wait commands", setupSyncWait in
# CoreV2/V3GenImpl). Tile's scheduler routinely attaches several. Post-pass:
# hoist all but the last wait of each instruction onto same-engine NOPs
# inserted immediately before it (same basic block, so per-engine program
# order is preserved).
def split_multiwaits(nc):
    for f in nc.m.functions:
        for blk in f.blocks:
            new = []
            changed = False
            for ins in blk.instructions:
                si = ins.sync_info
                if si is not None and len(si.on_wait) > 1:
                    waits = list(si.on_wait)
                    for w in waits[:-1]:
                        nop = mybir.InstNoOp(
                            name=nc.get_next_instruction_name(),
                            engine=ins.engine,
                            bass_nofuse=True,
                            sync_info=mybir.SyncInfo(on_wait=[w],
                                                     on_update=[]),
                        )
                        new.append(nop)
                    ins.sync_info = bass_rust.SyncInfo(
                        on_wait=[waits[-1]], on_update=list(si.on_update))
                    changed = True
                new.append(ins)
            if changed:
                blk.instructions = new


# ---------------------------------------------------------------------------
def _pair(ap, step_elems, nblocks=2):
    """Turn a contiguous [P, L] AP into [P, nblocks, L] with the given
    element step between blocks."""
    c = ap.copy()
    v = c.ap
    last = v.pop()
    v.append((step_elems, nblocks))
    v.append(tuple(last))
    c.ap = v
    return c


def build_bass(split=True):
    MULT = mybir.AluOpType.mult
    ADD = mybir.AluOpType.add

    nc = bass.Bass()
    pin = nc.dram_tensor("pin", [XS, CIN], F16, kind="ExternalInput").ap()
    pscal = nc.dram_tensor("pscal", [XS, NSCAL], F16,
                           kind="ExternalInput").ap()
    pout = nc.dram_tensor("pout", [XS, COUT], F16, kind="ExternalOutput").ap()

    with tile.TileContext(nc) as tc:
        with tc.tile_pool(name="m", bufs=1) as pool, \
             tc.psum_pool(name="p", bufs=4) as ppool:
            fF = pool.tile([XS, 2 * NS * NV], F16, tag="fF")
            fD0 = pool.tile([XS, 8 * NV], F16, tag="fD0")
            fW = pool.tile([XS, NDIAG * 128], F16, tag="fW")
            scal = pool.tile([XS, NSCAL], F16, tag="scal")
            P = pool.tile([XS, 2 * 6 * NV], F16, tag="P")
            G = pool.tile([XS, 2 * 4 * NV], F16, tag="G")
            OUT = pool.tile([XS, 2 * NS * NV], F16, tag="OUT")

            def fslot(k, imag, n=1):
                m = S[k][0]
                o = FOFF[m] + (imag * RL[m] + (k - RUN[m])) * NV
                return fF[:, o:o + n * NV]

            def fr(k):
                return fslot(k, 0)

            def fi(k):
                return fslot(k, 1)

            def pr(j):
                return P[:, j * NV:(j + 1) * NV]

            def pi(j):
                return P[:, (6 + j) * NV:(7 + j) * NV]

            def W(j):
                return fW[:, j * 128:(j + 1) * 128]

            def outr(k, n=1):
                m = S[k][0]
                o = FOFF[m] + (k - RUN[m]) * NV
                return OUT[:, o:o + n * NV]

            def outi(k, n=1):
                m = S[k][0]
                o = FOFF[m] + (RL[m] + k - RUN[m]) * NV
                return OUT[:, o:o + n * NV]

            def sc(col):
                return scal[:, col:col + 1]

            # ---- input DMAs: issued from the GpSimd queue, which is live
            # ~6us before the sync sequencer finishes its preamble; FIFO
            # drain makes emission order the arrival priority.
            # tiny scal table rides the sync queue (live by ~8.6us, needed
            # ~19us) so it doesn't burn a Q7 descriptor-gen slot at the head
            # of the input stream
            nc.sync.dma_start(scal[:], pscal[:])
            nc.gpsimd.dma_start(fW[:], pin[:, WOFF:WOFF + NDIAG * 128])

            def in_run(m):
                o = FOFF[m]
                n = 2 * RL[m] * NV
                nc.gpsimd.dma_start(fF[:, o:o + n], pin[:, o:o + n])

            in_run(0)
            in_run(1)
            in_run(2)
            nc.gpsimd.dma_start(fD0[:], pin[:, 28 * NV:36 * NV])
            in_run(3)
            in_run(4)

            # ---- DVE prescales ----
            def presc_G(il):        # (Gr,Gi) = 2*cB0(il) * (Fr1,Fi1)
                k1 = SIDX[(1, il)]
                return nc.vector.tensor_scalar_mul(
                    _pair(G[:, (il - 1) * NV:il * NV], 4 * NV),
                    _pair(fr(k1), RL[1] * NV),
                    float(-il * (il + 1)))

            def presc_SF(j):        # P = 2cB * F[im+1]   (fp16 TS at 4x)
                il, im = CB_PAIRS[j]
                ks = SIDX[(im + 1, il)]
                return nc.vector.tensor_scalar_mul(
                    _pair(pr(j), 6 * NV),
                    _pair(fr(ks), RL[im + 1] * NV),
                    2.0 * _cB(il, im))

            def presc_P(j):         # P += F[im-1]          (fp16 TT at 2x)
                il, im = CB_PAIRS[j]
                kb = SIDX[(im - 1, il)]
                return nc.vector.tensor_tensor(
                    _pair(pr(j), 6 * NV),
                    _pair(pr(j), 6 * NV),
                    _pair(fr(kb), RL[im - 1] * NV),
                    ADD)

            # Tile's per-engine scheduler reorders by readiness, which can
            # push the group-3 P operand behind the long im=0 chain.  Chain
            # the DVE ops explicitly so arrival-critical prescales run in
            # priority order.
            from bass_rust import add_dep_helper
            _dve_prev = None

            def dve_chain(ins):
                nonlocal _dve_prev
                if _dve_prev is not None:
                    add_dep_helper(ins.ins, _dve_prev.ins,
                                   reason="DVE priority order")
                _dve_prev = ins
                return ins

            gr = G[:, 0:4 * NV]
            gi = G[:, 4 * NV:8 * NV]
            d0i = fD0[:, 4 * NV:8 * NV]
            for il in range(1, L1):
                dve_chain(presc_G(il))  # needs run 1; feeds the b0 PE chain
            dve_chain(nc.vector.scalar_tensor_tensor(  # Di0 = D0i+0.5b1*Gi
                outi(0, 4), gi, sc(H1), d0i, MULT, ADD))
            for j in (0, 1, 3):     # group-1 operands (need runs 0 & 2)
                dve_chain(presc_SF(j))
                dve_chain(presc_P(j))
            for j in (2, 4):        # group-2 (runs 1 & 3)
                dve_chain(presc_SF(j))
                dve_chain(presc_P(j))
            dve_chain(presc_SF(5))  # group-3 (runs 2 & 4)
            dve_chain(presc_P(5))
            dve_chain(nc.vector.scalar_tensor_tensor(  # ... + 0.5b2*Gr
                outi(0, 4), gr, sc(H2), outi(0, 4), MULT, ADD))

            # ---- PE: pair-level pipeline.  Each (g, il) slot-pair is one
            # 2-bank PSUM tile and a complete 6-matmul chain that closes
            # immediately, so banks never sit open waiting for late operands
            # and the ACT evacuation drain starts ~15us earlier.  Emission
            # order = data-arrival order: diagonal pairs (no P operand)
            # first, then im=0 real row, then the P-merged middle pairs.
            def mm(bank, j, rhs, start=False, stop=False):
                nc.tensor.matmul(bank, W(j), rhs, start=start, stop=stop,
                                 skip_group_check=True)

            def pair_tile(name):
                return ppool.tile([XS, 2 * NV], F32, tag="pk", name=name)

            def pout_pair(k):
                m = S[k][0]
                o = FOFF[m] + (k - RUN[m]) * NV
                return _pair(pout[:, o:o + NV], RL[m] * NV)

            def diag_pair(g):
                pk = pair_tile(f"d{g}")
                bR, bI = pk[:, 0:NV], pk[:, NV:2 * NV]
                kp, ks = SIDX[(g - 1, g)], SIDX[(g, g)]
                mm(bR, DG_D1, fr(kp), start=True)
                mm(bI, DG_D1, fi(kp), start=True)
                mm(bR, DG_D3, fi(kp))
                mm(bI, DG_D2, fr(kp))
                mm(bR, DG_AP(g), fi(ks), stop=True)
                mm(bI, DG_AN(g), fr(ks), stop=True)
                nc.scalar.copy(_pair(outr(ks), RL[g] * NV), pk[:])
                nc.sync.dma_start(pout_pair(ks), _pair(outr(ks), RL[g] * NV))

            def mid_pair(il, g, evac="act"):
                pk = pair_tile(f"m{il}{g}")
                bR, bI = pk[:, 0:NV], pk[:, NV:2 * NV]
                ks = SIDX[(g, il)]
                j = CB_PAIRS.index((il, g))
                mm(bR, DG_AP(g), fi(ks), start=True)
                mm(bI, DG_AN(g), fr(ks), start=True)
                mm(bR, DG_D1, pr(j))
                mm(bI, DG_D1, pi(j))
                mm(bR, DG_D3, pi(j), stop=True)
                mm(bI, DG_D2, pr(j), stop=True)
                dst = _pair(outr(ks), RL[g] * NV)
                if evac == "act":
                    nc.scalar.copy(dst, pk[:])
                else:
                    dve_chain(nc.vector.tensor_copy(dst, pk[:]))
                nc.sync.dma_start(pout_pair(ks), dst)

            def b0_pairs():
                # im=0 real row: Dr0 = 1.5b1@Gr + 0.5b2@Gi + ones@D0r
                b0p = [pair_tile("b0a"), pair_tile("b0b")]

                def bk(il):
                    return b0p[(il - 1) // 2][:, ((il - 1) % 2) * NV:
                                              ((il - 1) % 2 + 1) * NV]

                for il in range(1, L1):
                    mm(bk(il), DG_D6, G[:, (il - 1) * NV:il * NV],
                       start=True)
                for il in range(1, L1):
                    mm(bk(il), DG_D2, G[:, (3 + il) * NV:(4 + il) * NV])
                for il in range(1, L1):
                    mm(bk(il), DG_ONES, fD0[:, (il - 1) * NV:il * NV],
                       stop=True)
                nc.scalar.copy(OUT[:, 0:2 * NV], b0p[0][:])
                nc.sync.dma_start(pout[:, 0:2 * NV], OUT[:, 0:2 * NV])
                nc.scalar.copy(OUT[:, 2 * NV:4 * NV], b0p[1][:])
                nc.sync.dma_start(pout[:, 2 * NV:4 * NV],
                                  OUT[:, 2 * NV:4 * NV])

            def out_run(m):
                o = FOFF[m]
                n = 2 * RL[m] * NV
                nc.sync.dma_start(pout[:, o:o + n], OUT[:, o:o + n])

            # PE warm-up: throwaway matmuls gated only on the early fW DMA
            # keep the HAM activity window busy so the real chains issue at
            # the warm 2.4GHz clock instead of the cold 1.2GHz default.
            warm = pair_tile("warm")
            for _ in range(20):
                mm(warm[:, 0:256], DG_D1, fW[:, 0:256], start=True,
                   stop=True)

            diag_pair(1)
            diag_pair(2)
            diag_pair(3)
            b0_pairs()
            mid_pair(2, 1)
            mid_pair(3, 1)
            mid_pair(4, 1)
            diag_pair(4)
            # im=0 imaginary half (DVE chain) — emitted before the late
            # mid-pair DMAs so the in-order sync queue cannot head-block it
            nc.sync.dma_start(pout[:, 4 * NV:8 * NV], OUT[:, 4 * NV:8 * NV])
            mid_pair(3, 2)
            mid_pair(4, 2)
            mid_pair(4, 3, "dve")

    if split:
        split_multiwaits(nc)
    return nc


# ---------------------------------------------------------------------------
def pack_inputs(prev_f_re, prev_f_im, delta0_re, delta0_im, b):
    """-> list of per-core {'pin': [XS, CIN] f16, 'pscal': [XS, 4] f32}."""
    pr = np.asarray(prev_f_re, np.float32)
    pi = np.asarray(prev_f_im, np.float32)
    d0r = np.asarray(delta0_re, np.float32)
    d0i = np.asarray(delta0_im, np.float32)
    bb = np.asarray(b, np.float32)
    ar = np.arange(XS)
    in_maps = []
    for c in range(NCORES):
        X = slice(c * XS, (c + 1) * XS)
        p = np.zeros((XS, CIN), np.float16)
        for k, (im, il) in enumerate(S):
            o = FOFF[im] + (k - RUN[im]) * NV
            p[:, o:o + NV] = pr[il, im, X, :]
            o += RL[im] * NV
            p[:, o:o + NV] = pi[il, im, X, :]
        for il in range(1, L1):
            p[:, (28 + il - 1) * NV:(29 + il - 1) * NV] = d0r[il, 0, X, :]
            p[:, (32 + il - 1) * NV:(33 + il - 1) * NV] = d0i[il, 0, X, :]
        b0, b1, b2 = bb[X, 0], bb[X, 1], bb[X, 2]
        diags = [0.5 * b1, 0.5 * b2, -0.5 * b2,
                 1.0 * b0, 2.0 * b0, 3.0 * b0, 4.0 * b0,
                 -1.0 * b0, -2.0 * b0, -3.0 * b0, -4.0 * b0,
                 np.ones(XS, np.float32), 1.5 * b1]
        for j, cx in enumerate(diags):
            blk = np.zeros((XS, 128), np.float16)
            blk[ar, ar] = cx.astype(np.float16)
            p[:, WOFF + j * 128:WOFF + (j + 1) * 128] = blk
        ps = np.zeros((XS, NSCAL), np.float16)
        ps[:, H1] = 0.5 * b1
        ps[:, H2] = 0.5 * b2
        in_maps.append({"pin": p, "pscal": ps})
    return in_maps


def unpack_outputs(results, delta0_re, delta0_im):
    out = np.zeros((L1, L1, NX, NV), np.complex64)
    out[0, 0] = np.asarray(delta0_re[0, 0]) + 1j * np.asarray(delta0_im[0, 0])
    for c in range(NCORES):
        X = slice(c * XS, (c + 1) * XS)
        p = results[c]["pout"]
        for k, (im, il) in enumerate(S):
            o = FOFF[im] + (k - RUN[im]) * NV
            dr = p[:, o:o + NV].astype(np.float32)
            o += RL[im] * NV
            di = p[:, o:o + NV].astype(np.float32)
            out[il, im, X, :] = dr + 1j * di
    return out


_NC_CACHE = None


def get_nc():
    global _NC_CACHE
    if _NC_CACHE is None:
        _NC_CACHE = build_bass()
    return _NC_CACHE


def kernel(prev_f_re, prev_f_im, delta0_re, delta0_im, b, v):
    in_maps = pack_inputs(prev_f_re, prev_f_im, delta0_re, delta0_im, b)
    res = run_bass_kernel_spmd(get_nc(), in_maps, list(range(NCORES)))
    return unpack_outputs(res.results, delta0_re, delta0_im)
